# revision 1
# baseline (speedup 1.0000x reference)
"""Trainium2 Bass kernel for the labelled contrastive loss.

Math (per batch row b, label L, over C=200 centers):
    cos[b,c] = <f_b, c_c> / (|f_b| |c_c|)
    a = |cos|;  l1_b = sum_c a[b,c];  row term = (2*a[b,L_b] - l1_b)/l1_b
    loss = -sum over labelled rows of row term
The feature norm |f_b| cancels in the ratio, so the kernel never computes
it: it works on raw = f @ cn^T with cn = centers/max(|c|,eps) normalized on
host (O(C*D), negligible), and forms (2*T - S)/S with
    S = sum_c |raw|,  T = |raw[b, L_b]|.

Sharding: data-parallel over the batch axis, 4096 rows per core across
8 cores; centers replicated. Per-core output is a [128,1] vector of
per-partition partial sums; the host adds them up and negates.

Device pipeline, two 128-row tiles ("a pair") at a time:
    DMA   : feature chunks [128d x 2 x 6 x 128b] (host pre-transposed so the
            contraction dim is on partitions -- no on-chip transposes)
    PE    : 2x6 accumulating matmuls (bf16 in, f32 PSUM) -> cos pair
            [128b, 2, 200c] in a single PSUM bank
    ACT   : per tile, Abs with accum_out -> exact f32 S column (the |cos|
            output itself is a throwaway; only the accumulator is used)
    DVE   : one-hot mask = is_equal(iota, label broadcast); signed
            T = rowsum(cos * mask), batched over the pair; f32 throughout
Epilogue on [128, 32] f32 tiles: T=|T|; msk * (2T - S)/S; row-reduce; DMA.

bf16 is used only for the matmul inputs; S is accumulated in f32 from the
f32 PSUM and the final ratio is f32, so input rounding enters the per-row
term only at second order (measured ~1e-7 relative on the final scalar).
"""

import numpy as np
import ml_dtypes

import concourse.bass as bass
import concourse.tile as tile
from concourse import mybir
from concourse.bass_utils import run_bass_kernel_spmd

# ---------------------------------------------------------------------------
# Workaround for walrus "Too many sync wait commands": this toolchain only
# encodes a limited number of sem waits per instruction, so spread excess
# waits over preceding same-engine nops — both for scheduled instructions
# (pre-lowering pass) and for the TileContext tail drain.
# ---------------------------------------------------------------------------
from concourse.vector_clock import ScopedClock

_MAX_WAITS = 1
_split_counter = [0]


def _split_waits_in_ordered(ordered):
    for bb_name, insts in ordered.items():
        new = []
        for inst in insts:
            si = getattr(inst, "sync_info", None)
            waits = list(si.on_wait) if si is not None and si.on_wait else []
            if len(waits) > _MAX_WAITS:
                updates = list(si.on_update) if si.on_update else []
                head, tail = waits[:-_MAX_WAITS], waits[-_MAX_WAITS:]
                while head:
                    n = mybir.InstNoOp(
                        name=f"I-wsplit-{_split_counter[0]}", ins=[], outs=[]
                    )
                    _split_counter[0] += 1
                    n.engine = inst.engine
                    n.bass_nofuse = True
                    n.sync_info = mybir.SyncInfo(
                        on_wait=head[:_MAX_WAITS], on_update=[]
                    )
                    head = head[_MAX_WAITS:]
                    new.append(n)
                inst.sync_info = mybir.SyncInfo(on_wait=tail, on_update=updates)
            new.append(inst)
        ordered[bb_name] = new


_orig_lower_ordered = tile.TileContext._lower_ordered_insts


def _patched_lower_ordered(self, ordered):
    _split_waits_in_ordered(ordered)
    return _orig_lower_ordered(self, ordered)


tile.TileContext._lower_ordered_insts = _patched_lower_ordered


def _patched_drain_and_barrier(self, tick_clock, wait_clock):
    """Minimal kernel tail replacing the stock drain + two EVSEM-butterfly
    barriers (~15us):

    1. SP nops carry one sem wait each for every proc's final clock tick —
       once they pass, every tracked semaphore increment has LANDED (waits
       observe the final value of each proc's latest sem; same-engine and
       same-queue increments retire in order).
    2. Each engine drains its pipeline and bumps a tail semaphore; once it
       passes its own last wait nothing can block it, so this retires.
    3. GpSimd waits for the 4 other engines + SP, then range-clears all
       tile semaphores, resets DMA queue state and clears the tail sem.
    4. Engines halt independently; the NEFF only completes (and can only
       be re-executed) when every engine including GpSimd has halted, so
       the next run starts with everything zeroed.
    """
    nc = self.nc
    carrier = nc.sync.nop(nofuse=True)
    wait_clock.add_sem_waits(carrier.ins, ScopedClock({None: tick_clock.global_clock}))
    si = carrier.ins.sync_info
    waits = list(si.on_wait) if si is not None and si.on_wait else []
    if len(waits) > _MAX_WAITS:
        updates = list(si.on_update) if si.on_update else []
        carrier.ins.sync_info = mybir.SyncInfo(on_wait=[], on_update=updates)
        rest = waits
        while rest:
            n = nc.sync.nop(nofuse=True)
            n.ins.sync_info = mybir.SyncInfo(on_wait=rest[:_MAX_WAITS], on_update=[])
            rest = rest[_MAX_WAITS:]
    nc.sync.drain()

    tail_sem = nc.alloc_semaphore("tile_tail_sem")
    n_inc = 0
    for eng_type, eng in nc.engines.items():
        if eng_type == mybir.EngineType.Pool:
            continue
        eng.drain()
        eng.sem_inc(tail_sem, 1)
        n_inc += 1
    nc.gpsimd.drain()
    nc.gpsimd.wait_ge(tail_sem, n_inc)

    assert self.sems is not None
    popped = nc._tile_sem_poison_stack.pop()
    assert popped is self._sem_poison
    nc.clear_and_free_semaphores(list(self.sems.allocated().values()))
    nc.clear_and_free_semaphores([tail_sem])


tile.TileContext._drain_and_barrier = _patched_drain_and_barrier

# ---------------------------------------------------------------------------
# Problem constants (hardcoded per contract)
# ---------------------------------------------------------------------------
N_CORES = 8
B, D, C = 32768, 768, 200
B_CORE = B // N_CORES          # 4096
P = 128                        # partitions
KCH = D // P                   # 6 contraction chunks
NT = B_CORE // P               # 32 tiles per core
NPAIR = NT // 2                # 16 pairs
CP = 256                       # padded per-tile PSUM pitch (bank alignment)
EPS_COS = 1e-8

_TRACE = False                 # test.py flips this for profiling runs
_TRACE_DIR = None
last_results = None

_nc = None


def _build():
    global _nc
    if _nc is not None:
        return _nc
    nc = bass.Bass("TRN2", debug=False, num_devices=N_CORES)

    bf16 = mybir.dt.bfloat16
    f32 = mybir.dt.float32

    # ft[quad, p, t', k, b] = features[(4*quad+t')*128 + b, k*128 + p], bf16
    ft = nc.dram_tensor("ft", [NPAIR, P, 2, KCH, P], bf16, kind="ExternalInput")
    cnt = nc.dram_tensor("cnt", [P, KCH, C], bf16, kind="ExternalInput")
    iota = nc.dram_tensor("iota", [P, 4, C], f32, kind="ExternalInput")
    lab = nc.dram_tensor("lab", [P, NT], f32, kind="ExternalInput")
    msk = nc.dram_tensor("msk", [P, NT], f32, kind="ExternalInput")
    out = nc.dram_tensor("out", [1, 1], f32, kind="ExternalOutput")

    with tile.TileContext(nc) as tc:
        with (
            tc.tile_pool(name="singles", bufs=1) as singles,
            tc.tile_pool(name="ftp", bufs=8) as ftp,
            tc.tile_pool(name="maskp", bufs=8) as maskp,
            tc.tile_pool(name="work", bufs=4) as work,
            tc.tile_pool(name="psum", bufs=4, space="PSUM") as psum,
            tc.tile_pool(name="psum1", bufs=1, space="PSUM") as psum1,
        ):
            cnt_sb = singles.tile([P, KCH, C], bf16)
            nc.sync.dma_start(cnt_sb[:], cnt[:])
            iota_sb = singles.tile([P, 4, C], f32)
            nc.sync.dma_start(iota_sb[:], iota[:])
            lab_sb = singles.tile([P, NT], f32)
            nc.sync.dma_start(lab_sb[:], lab[:])
            msk_sb = singles.tile([P, NT], f32)
            nc.sync.dma_start(msk_sb[:], msk[:])

            s_all = singles.tile([P, NT], f32)
            t_all = singles.tile([P, NT], f32)

            for pr in range(NPAIR):
                t0 = 2 * pr
                ft_sb = ftp.tile([P, 2, KCH, P], bf16)
                nc.sync.dma_start(ft_sb[:], ft[pr])

                # one-hot masks for 4 tiles at a time (2 pairs)
                if pr % 2 == 0:
                    mask_sb = maskp.tile([P, 4, C], f32, tag="mask")
                    nc.vector.tensor_tensor(
                        out=mask_sb[:],
                        in0=iota_sb[:],
                        in1=lab_sb[:, t0 : t0 + 4].broadcast_to([P, 4, C]),
                        op=mybir.AluOpType.is_equal,
                    )
                mhalf = (pr % 2) * 2

                cos_ps = psum.tile([P, 2, C], f32)
                for j in range(2):
                    for k in range(KCH):
                        nc.tensor.matmul(
                            cos_ps[:, j, :],
                            ft_sb[:, j, k, :],
                            cnt_sb[:, k, :],
                            start=(k == 0),
                            stop=(k == KCH - 1),
                        )

                # S columns: ACT Abs with row-sum accumulator (out is junk)
                junk_sb = work.tile([P, 2, C], bf16, tag="junk")
                for j in range(2):
                    nc.scalar.activation(
                        out=junk_sb[:, j, :],
                        in_=cos_ps[:, j, :],
                        func=mybir.ActivationFunctionType.Abs,
                        accum_out=s_all[:, t0 + j : t0 + j + 1],
                    )

                # signed T columns for the pair on DVE (f32)
                am_sb = work.tile([P, 2, C], f32, tag="am")
                nc.vector.tensor_tensor(
                    out=am_sb[:], in0=cos_ps[:],
                    in1=mask_sb[:, mhalf : mhalf + 2, :],
                    op=mybir.AluOpType.mult,
                )
                nc.vector.tensor_reduce(
                    out=t_all[:, t0 : t0 + 2], in_=am_sb[:],
                    op=mybir.AluOpType.add, axis=mybir.AxisListType.X,
                )

            # epilogue: T = |T|; per-row term = msk * (2*T - S) / S; reduce
            t_abs = singles.tile([P, NT], f32)
            nc.scalar.activation(
                out=t_abs[:], in_=t_all[:],
                func=mybir.ActivationFunctionType.Abs,
            )
            recip = singles.tile([P, NT], f32)
            nc.vector.reciprocal(recip[:], s_all[:])
            num = singles.tile([P, NT], f32)
            nc.vector.tensor_scalar(
                out=num[:],
                in0=t_abs[:],
                scalar1=2.0,
                scalar2=None,
                op0=mybir.AluOpType.mult,
            )
            nc.vector.tensor_tensor(
                out=num[:], in0=num[:], in1=s_all[:], op=mybir.AluOpType.subtract
            )
            nc.vector.tensor_tensor(
                out=num[:], in0=num[:], in1=recip[:], op=mybir.AluOpType.mult
            )
            nc.vector.tensor_tensor(
                out=num[:], in0=num[:], in1=msk_sb[:], op=mybir.AluOpType.mult
            )
            # collapse to one scalar on-chip: PE sums over partitions, DVE
            # over the NT columns -- so the store is a single 4B descriptor
            # (a [128,1] store would spray 128 tiny descriptors over all 16
            # DMA engines, whose completion events straggle for ~6us).
            ones_sb = singles.tile([P, 1], f32)
            nc.vector.memset(ones_sb[:], 1.0)
            tot_ps = psum1.tile([1, NT], f32)
            nc.tensor.matmul(tot_ps[:], ones_sb[:], num[:], start=True, stop=True)
            out_sb = singles.tile([1, 1], f32)
            nc.vector.tensor_reduce(
                out=out_sb[:], in_=tot_ps[:], op=mybir.AluOpType.add,
                axis=mybir.AxisListType.X,
            )
            nc.sync.dma_start(out[:], out_sb[:])

    _nc = nc
    return nc


def kernel(features, centers, labels, labelled_or_not):
    global last_results
    nc = _build()

    bf = ml_dtypes.bfloat16
    features = np.asarray(features, dtype=np.float32)
    centers = np.asarray(centers, dtype=np.float32)
    labels_f = np.asarray(labels).astype(np.float32)
    msk_f = np.asarray(labelled_or_not).astype(np.float32)

    # normalized + transposed centers -> [P, KCH, C] in bf16
    cn = centers / np.maximum(
        np.linalg.norm(centers, axis=1, keepdims=True), EPS_COS
    )
    cnt_host = np.ascontiguousarray(
        cn.reshape(C, KCH, P).transpose(2, 1, 0).astype(bf)
    )
    iota_host = np.ascontiguousarray(
        np.broadcast_to(np.arange(C, dtype=np.float32), (P, 4, C))
    )

    in_maps = []
    for c in range(N_CORES):
        sl = slice(c * B_CORE, (c + 1) * B_CORE)
        fcore = features[sl]  # [4096, 768]
        # ft[pair, p, t', k, b] = f[(2*pair+t')*128 + b, k*128 + p]
        ft_host = np.ascontiguousarray(
            fcore.reshape(NPAIR, 2, P, KCH, P).transpose(0, 4, 1, 3, 2).astype(bf)
        )
        lab_host = np.ascontiguousarray(labels_f[sl].reshape(NT, P).T)
        msk_host = np.ascontiguousarray(msk_f[sl].reshape(NT, P).T)
        in_maps.append(
            {
                "ft": ft_host,
                "cnt": cnt_host,
                "iota": iota_host,
                "lab": lab_host,
                "msk": msk_host,
            }
        )

    kwargs = {}
    if _TRACE:
        kwargs["trace"] = True
        if _TRACE_DIR:
            kwargs["tmpdir"] = _TRACE_DIR
    res = run_bass_kernel_spmd(nc, in_maps, core_ids=list(range(N_CORES)), **kwargs)
    last_results = res

    total = 0.0
    for c in range(N_CORES):
        total += float(res.results[c]["out"][0, 0])
    return np.array(-total, dtype=np.float32)



# revision 10
# speedup vs baseline: 1.5931x; 1.5931x over previous
"""Trainium2 Bass kernel for the labelled contrastive loss.

Math (per batch row b, label L, over C=200 centers):
    cos[b,c] = <f_b, c_c> / (|f_b| |c_c|)
    a = |cos|;  S_b = sum_c a[b,c];  row term = (2*a[b,L_b] - S_b)/S_b
    loss = -sum over labelled rows of row term
Two exact host-side reductions shrink the device problem:
  1. |f_b| cancels in the ratio -> never computed; centers are normalized
     on host (O(C*D)) and the kernel works on raw = f @ cn^T.
  2. loss = N_labelled - sum_labelled 2*T_b/S_b, and UNLABELLED ROWS
     CONTRIBUTE NOTHING -> the kernel only processes the ~B/2 labelled
     rows; N_labelled is counted on host.

Device layout tricks:
  - fp8(e4m3) features/centers with perf_mode=DoubleRow matmuls (2 k-planes
    per pass, fp8 double-pumped PE). C padded 200->208 zero centers so the
    DoubleRow moving-AP stride is 16B-aligned; the pad columns produce
    cos=0 which add nothing to S.
  - Labelled rows are SORTED BY LABEL on host and each core's centers are
    ROTATED so that core's label range maps to columns [0, 32). T-extraction
    is then one batched 32-wide masked reduce per PSUM quad instead of a
    200-wide one-hot per tile; the mask (with 2*labelled folded in) is host
    precomputed. Zero-pad rows duplicate the last real row (finite S) with
    mask 0.
  - Per 4-tile PSUM quad (2 banks): 12 DoubleRow matmuls -> one ACT Abs
    (f32 PSUM -> bf16 SBUF) -> DVE rowsum for S + masked window reduce for
    T'. Epilogue: T'*(1/S), ones-matmul partition collapse, single 4B store.
"""

import math

import numpy as np
import ml_dtypes

import concourse.bass as bass
import concourse.tile as tile
from concourse import mybir
from concourse.bass_utils import run_bass_kernel_spmd

# ---------------------------------------------------------------------------
# Workaround for walrus "Too many sync wait commands": this toolchain only
# encodes a limited number of sem waits per instruction, so spread excess
# waits over preceding same-engine nops — both for scheduled instructions
# (pre-lowering pass) and for the TileContext tail drain.
# ---------------------------------------------------------------------------
from concourse.vector_clock import ScopedClock

_MAX_WAITS = 1
_split_counter = [0]


def _split_waits_in_ordered(ordered):
    for bb_name, insts in ordered.items():
        new = []
        for inst in insts:
            si = getattr(inst, "sync_info", None)
            waits = list(si.on_wait) if si is not None and si.on_wait else []
            if len(waits) > _MAX_WAITS:
                updates = list(si.on_update) if si.on_update else []
                head, tail = waits[:-_MAX_WAITS], waits[-_MAX_WAITS:]
                while head:
                    n = mybir.InstNoOp(
                        name=f"I-wsplit-{_split_counter[0]}", ins=[], outs=[]
                    )
                    _split_counter[0] += 1
                    n.engine = inst.engine
                    n.bass_nofuse = True
                    n.sync_info = mybir.SyncInfo(
                        on_wait=head[:_MAX_WAITS], on_update=[]
                    )
                    head = head[_MAX_WAITS:]
                    new.append(n)
                inst.sync_info = mybir.SyncInfo(on_wait=tail, on_update=updates)
            new.append(inst)
        ordered[bb_name] = new


_orig_lower_ordered = tile.TileContext._lower_ordered_insts


def _patched_lower_ordered(self, ordered):
    _split_waits_in_ordered(ordered)
    return _orig_lower_ordered(self, ordered)


tile.TileContext._lower_ordered_insts = _patched_lower_ordered

_ORIG_DRAIN = tile.TileContext._drain_and_barrier


def _patched_drain_and_barrier(self, tick_clock, wait_clock):
    """Minimal kernel tail replacing the stock drain + two EVSEM-butterfly
    barriers (~15us). See the original notes: SP nops carry the final sem
    waits, engines drain + bump a tail semaphore, GpSimd waits for them and
    range-clears tile semaphores/queue state; halting engines complete the
    NEFF with everything zeroed for re-execution.
    """
    nc = self.nc
    carrier = nc.sync.nop(nofuse=True)
    wait_clock.add_sem_waits(carrier.ins, ScopedClock({None: tick_clock.global_clock}))
    si = carrier.ins.sync_info
    waits = list(si.on_wait) if si is not None and si.on_wait else []
    if len(waits) > _MAX_WAITS:
        updates = list(si.on_update) if si.on_update else []
        carrier.ins.sync_info = mybir.SyncInfo(on_wait=[], on_update=updates)
        rest = waits
        while rest:
            n = nc.sync.nop(nofuse=True)
            n.ins.sync_info = mybir.SyncInfo(on_wait=rest[:_MAX_WAITS], on_update=[])
            rest = rest[_MAX_WAITS:]
    nc.sync.drain()

    tail_sem = nc.alloc_semaphore("tile_tail_sem")
    n_inc = 0
    for eng_type, eng in nc.engines.items():
        if eng_type == mybir.EngineType.Pool:
            continue
        eng.drain()
        eng.sem_inc(tail_sem, 1)
        n_inc += 1
    nc.gpsimd.drain()
    nc.gpsimd.wait_ge(tail_sem, n_inc)

    assert self.sems is not None
    popped = nc._tile_sem_poison_stack.pop()
    assert popped is self._sem_poison
    nc.clear_and_free_semaphores(list(self.sems.allocated().values()))
    nc.clear_and_free_semaphores([tail_sem])


tile.TileContext._drain_and_barrier = _patched_drain_and_barrier

# ---------------------------------------------------------------------------
# Problem constants (hardcoded per contract)
# ---------------------------------------------------------------------------
N_CORES = 8
B, D, C = 32768, 768, 200
P = 128                        # partitions
KP = 3                         # DoubleRow k passes (each contracts 256 of 768)
CPAD = 208                     # padded center count (16B-aligned fp8 stride)
W = 32                         # label window width after per-core rotation
EPS_COS = 1e-8

_TRACE = False                 # test.py flips this for profiling runs
_TRACE_DIR = None
last_results = None

_built = {}                    # nt -> Bass


def _groups(nt):
    gs = [4] * (nt // 4)
    if nt % 4:
        gs.append(nt % 4)
    return gs


def _build(nt):
    """Device program for nt 128-row tiles per core."""
    if nt in _built:
        return _built[nt]
    nc = bass.Bass("TRN2", debug=False, num_devices=N_CORES)

    fp8 = mybir.dt.float8e4
    bf16 = mybir.dt.bfloat16
    f32 = mybir.dt.float32
    DR = mybir.MatmulPerfMode.DoubleRow

    # ft[p, t, s, b] = f_rows[t*128 + b, s*128 + p], fp8 (partition-major so
    # group DMAs iterate src and dst in the same order)
    ft = nc.dram_tensor("ft", [P, nt, 2 * KP, P], fp8, kind="ExternalInput")
    # cnt[p, s, c] = cn_rot[c, s*128 + p], fp8 (c in [200,208) zero)
    cnt = nc.dram_tensor("cnt", [P, 2 * KP, CPAD], fp8, kind="ExternalInput")
    # mask[p, t, j] = 2*labelled at the row's rotated label column, bf16
    msk = nc.dram_tensor("msk", [P, nt, W], bf16, kind="ExternalInput")
    out = nc.dram_tensor("out", [1, 1], f32, kind="ExternalOutput")

    with tile.TileContext(nc) as tc:
        with (
            tc.tile_pool(name="singles", bufs=1) as singles,
            tc.tile_pool(name="ftp", bufs=3) as ftp,
            tc.tile_pool(name="work", bufs=2) as work,
            tc.tile_pool(name="psum", bufs=3, space="PSUM") as psum,
            tc.tile_pool(name="psum1", bufs=1, space="PSUM") as psum1,
        ):
            cnt_sb = singles.tile([P, 2 * KP, CPAD], fp8)
            nc.sync.dma_start(cnt_sb[:], cnt[:])
            msk_sb = singles.tile([P, nt, W], bf16)
            nc.sync.dma_start(msk_sb[:], msk[:])

            abs_all = singles.tile([P, nt, CPAD], bf16)
            s_all = singles.tile([P, nt], f32)
            t2_all = singles.tile([P, nt], f32)

            t0 = 0
            for gsz in _groups(nt):
                ft_sb = ftp.tile([P, gsz, 2 * KP, P], fp8, tag="ft")
                nc.sync.dma_start(ft_sb[:], ft[:, t0 : t0 + gsz])

                # 256-elem pitch: each 208-wide tile slice stays in one bank
                cos_ps = psum.tile([P, gsz, 256], f32, tag="cos")
                for j in range(gsz):
                    for kq in range(KP):
                        nc.tensor.matmul(
                            cos_ps[:, j, 0:CPAD],
                            ft_sb[:, j, 2 * kq : 2 * kq + 2, :],
                            cnt_sb[:, 2 * kq : 2 * kq + 2, :],
                            start=(kq == 0),
                            stop=(kq == KP - 1),
                            perf_mode=DR,
                        )

                # |cos| group: f32 PSUM -> bf16 SBUF on ACT
                nc.scalar.activation(
                    out=abs_all[:, t0 : t0 + gsz, :],
                    in_=cos_ps[:, :, 0:CPAD],
                    func=mybir.ActivationFunctionType.Abs,
                )
                # S group: plain rowsum (pad columns are exactly 0)
                nc.vector.tensor_reduce(
                    out=s_all[:, t0 : t0 + gsz],
                    in_=abs_all[:, t0 : t0 + gsz, :],
                    op=mybir.AluOpType.add,
                    axis=mybir.AxisListType.X,
                )
                # T' group: masked 32-wide window reduce -> 2*labelled*|cos@label|
                tt = work.tile([P, gsz, W], bf16, tag="tt")
                nc.vector.tensor_tensor(
                    out=tt[:],
                    in0=abs_all[:, t0 : t0 + gsz, 0:W],
                    in1=msk_sb[:, t0 : t0 + gsz, :],
                    op=mybir.AluOpType.mult,
                )
                nc.vector.tensor_reduce(
                    out=t2_all[:, t0 : t0 + gsz],
                    in_=tt[:],
                    op=mybir.AluOpType.add,
                    axis=mybir.AxisListType.X,
                )
                t0 += gsz

            # epilogue: term = T' / S; collapse to one scalar on-chip
            recip = singles.tile([P, nt], f32)
            nc.vector.reciprocal(recip[:], s_all[:])
            term = singles.tile([P, nt], f32)
            nc.vector.tensor_tensor(
                out=term[:], in0=t2_all[:], in1=recip[:], op=mybir.AluOpType.mult
            )
            ones_sb = singles.tile([P, 1], f32)
            nc.vector.memset(ones_sb[:], 1.0)
            tot_ps = psum1.tile([1, nt], f32)
            nc.tensor.matmul(tot_ps[:], ones_sb[:], term[:], start=True, stop=True)
            out_sb = singles.tile([1, 1], f32)
            nc.vector.tensor_reduce(
                out=out_sb[:], in_=tot_ps[:], op=mybir.AluOpType.add,
                axis=mybir.AxisListType.X,
            )
            nc.sync.dma_start(out[:], out_sb[:])

    _built[nt] = nc
    return nc


def kernel(features, centers, labels, labelled_or_not):
    global last_results

    f8 = ml_dtypes.float8_e4m3
    bf = ml_dtypes.bfloat16
    features = np.asarray(features, dtype=np.float32)
    centers = np.asarray(centers, dtype=np.float32)
    labels_i = np.asarray(labels).astype(np.int64)
    lab_b = np.asarray(labelled_or_not).astype(bool)

    n_lab = int(lab_b.sum())
    assert n_lab > 0

    # keep only labelled rows; sort them by label
    f_l = features[lab_b]
    l_l = labels_i[lab_b]
    order = np.argsort(l_l, kind="stable")
    f_s = f_l[order]
    l_s = l_l[order]

    # near-equal per-core real-row counts (padding spread over all cores so
    # each core's label span stays well under W)
    base, rem = divmod(n_lab, N_CORES)
    counts = [base + (1 if c < rem else 0) for c in range(N_CORES)]
    nt = max(1, -(-counts[0] // P))
    rows_core = nt * P
    starts = np.cumsum([0] + counts)

    # normalized centers (host, O(C*D))
    cn = centers / np.maximum(
        np.linalg.norm(centers, axis=1, keepdims=True), EPS_COS
    )

    in_maps = []
    for c in range(N_CORES):
        nreal = counts[c]
        fc = f_s[starts[c] : starts[c] + nreal]
        lc = l_s[starts[c] : starts[c] + nreal]
        pad = rows_core - nreal
        if pad:
            # repeat last real row (finite S) with mask weight 0
            fc = np.concatenate([fc, np.repeat(fc[-1:], pad, axis=0)])
            lc = np.concatenate([lc, np.repeat(lc[-1:], pad)])
        wc = np.zeros(rows_core, dtype=np.float32)
        wc[:nreal] = 2.0

        # rotate centers so this core's labels land in columns [0, W)
        l0 = int(lc[0])
        rel = (lc - l0) % C
        assert rel.max() < W, f"core {c} label span {rel.max()} >= {W}"
        cn_rot = cn[(l0 + np.arange(C)) % C]
        cnt_full = np.zeros((CPAD, D), dtype=np.float32)
        cnt_full[:C] = cn_rot
        cnt_host = np.ascontiguousarray(
            cnt_full.reshape(CPAD, 2 * KP, P).transpose(2, 1, 0).astype(f8)
        )

        # ft[p, t, s, b] = fc[t*128 + b, s*128 + p]
        ft_host = np.ascontiguousarray(
            fc.reshape(nt, P, 2 * KP, P).transpose(3, 0, 2, 1).astype(f8)
        )

        # mask[p, t, j]: 2*labelled at the rotated label column
        mask_flat = np.zeros((rows_core, W), dtype=np.float32)
        mask_flat[np.arange(rows_core), rel] = wc
        msk_host = np.ascontiguousarray(
            mask_flat.reshape(nt, P, W).transpose(1, 0, 2).astype(bf)
        )

        in_maps.append({"ft": ft_host, "cnt": cnt_host, "msk": msk_host})

    nc = _build(nt)
    kwargs = {}
    if _TRACE:
        kwargs["trace"] = True
        if _TRACE_DIR:
            kwargs["tmpdir"] = _TRACE_DIR
    res = run_bass_kernel_spmd(nc, in_maps, core_ids=list(range(N_CORES)), **kwargs)
    last_results = res

    total = 0.0
    for c in range(N_CORES):
        total += float(res.results[c]["out"][0, 0])
    # loss = -sum_labelled (2T - S)/S = N_labelled - sum 2T/S
    return np.array(float(n_lab) - total, dtype=np.float32)


# revision 13
# speedup vs baseline: 1.6273x; 1.0215x over previous
"""Trainium2 Bass kernel for the labelled contrastive loss.

Math (per batch row b, label L, over C=200 centers):
    cos[b,c] = <f_b, c_c> / (|f_b| |c_c|)
    a = |cos|;  S_b = sum_c a[b,c];  row term = (2*a[b,L_b] - S_b)/S_b
    loss = -sum over labelled rows of row term
Two exact host-side reductions shrink the device problem:
  1. |f_b| cancels in the ratio -> never computed; centers are normalized
     on host (O(C*D)) and the kernel works on raw = f @ cn^T.
  2. loss = N_labelled - sum_labelled 2*T_b/S_b, and UNLABELLED ROWS
     CONTRIBUTE NOTHING -> the kernel only processes the ~B/2 labelled
     rows; N_labelled is counted on host.

Device layout tricks:
  - fp8(e4m3) features/centers with perf_mode=DoubleRow matmuls (2 k-planes
    per pass, fp8 double-pumped PE). C padded 200->208 zero centers so the
    DoubleRow moving-AP stride is 16B-aligned; the pad columns produce
    cos=0 which add nothing to S.
  - Labelled rows are SORTED BY LABEL on host and each core's centers are
    ROTATED so that core's label range maps to columns [0, 32). T-extraction
    is then one batched 32-wide masked reduce per PSUM quad instead of a
    200-wide one-hot per tile; the mask (with 2*labelled folded in) is host
    precomputed. Zero-pad rows duplicate the last real row (finite S) with
    mask 0.
  - Per 4-tile PSUM quad (2 banks): 12 DoubleRow matmuls -> one ACT Abs
    (f32 PSUM -> bf16 SBUF) -> DVE rowsum for S + masked window reduce for
    T'. Epilogue: T'*(1/S), ones-matmul partition collapse, single 4B store.
"""

import math

import numpy as np
import ml_dtypes

import concourse.bass as bass
import concourse.tile as tile
from concourse import mybir
from concourse.bass_utils import run_bass_kernel_spmd

# ---------------------------------------------------------------------------
# Workaround for walrus "Too many sync wait commands": this toolchain only
# encodes a limited number of sem waits per instruction, so spread excess
# waits over preceding same-engine nops — both for scheduled instructions
# (pre-lowering pass) and for the TileContext tail drain.
# ---------------------------------------------------------------------------
from concourse.vector_clock import ScopedClock

_MAX_WAITS = 1
_split_counter = [0]


def _split_waits_in_ordered(ordered):
    for bb_name, insts in ordered.items():
        new = []
        for inst in insts:
            si = getattr(inst, "sync_info", None)
            waits = list(si.on_wait) if si is not None and si.on_wait else []
            if len(waits) > _MAX_WAITS:
                updates = list(si.on_update) if si.on_update else []
                head, tail = waits[:-_MAX_WAITS], waits[-_MAX_WAITS:]
                while head:
                    n = mybir.InstNoOp(
                        name=f"I-wsplit-{_split_counter[0]}", ins=[], outs=[]
                    )
                    _split_counter[0] += 1
                    n.engine = inst.engine
                    n.bass_nofuse = True
                    n.sync_info = mybir.SyncInfo(
                        on_wait=head[:_MAX_WAITS], on_update=[]
                    )
                    head = head[_MAX_WAITS:]
                    new.append(n)
                inst.sync_info = mybir.SyncInfo(on_wait=tail, on_update=updates)
            new.append(inst)
        ordered[bb_name] = new


_orig_lower_ordered = tile.TileContext._lower_ordered_insts


def _patched_lower_ordered(self, ordered):
    _split_waits_in_ordered(ordered)
    return _orig_lower_ordered(self, ordered)


tile.TileContext._lower_ordered_insts = _patched_lower_ordered

_ORIG_DRAIN = tile.TileContext._drain_and_barrier


def _patched_drain_and_barrier(self, tick_clock, wait_clock):
    """Minimal kernel tail replacing the stock drain + two EVSEM-butterfly
    barriers (~15us). See the original notes: SP nops carry the final sem
    waits, engines drain + bump a tail semaphore, GpSimd waits for them and
    range-clears tile semaphores/queue state; halting engines complete the
    NEFF with everything zeroed for re-execution.
    """
    nc = self.nc
    carrier = nc.sync.nop(nofuse=True)
    wait_clock.add_sem_waits(carrier.ins, ScopedClock({None: tick_clock.global_clock}))
    si = carrier.ins.sync_info
    waits = list(si.on_wait) if si is not None and si.on_wait else []
    if len(waits) > _MAX_WAITS:
        updates = list(si.on_update) if si.on_update else []
        carrier.ins.sync_info = mybir.SyncInfo(on_wait=[], on_update=updates)
        rest = waits
        while rest:
            n = nc.sync.nop(nofuse=True)
            n.ins.sync_info = mybir.SyncInfo(on_wait=rest[:_MAX_WAITS], on_update=[])
            rest = rest[_MAX_WAITS:]
    nc.sync.drain()

    tail_sem = nc.alloc_semaphore("tile_tail_sem")
    n_inc = 0
    for eng_type, eng in nc.engines.items():
        if eng_type == mybir.EngineType.Pool:
            continue
        eng.drain()
        eng.sem_inc(tail_sem, 1)
        n_inc += 1
    nc.gpsimd.drain()
    nc.gpsimd.wait_ge(tail_sem, n_inc)

    assert self.sems is not None
    popped = nc._tile_sem_poison_stack.pop()
    assert popped is self._sem_poison
    nc.clear_and_free_semaphores(list(self.sems.allocated().values()))
    nc.clear_and_free_semaphores([tail_sem])


tile.TileContext._drain_and_barrier = _patched_drain_and_barrier

# ---------------------------------------------------------------------------
# Problem constants (hardcoded per contract)
# ---------------------------------------------------------------------------
N_CORES = 8
B, D, C = 32768, 768, 200
P = 128                        # partitions
KP = 3                         # DoubleRow k passes (each contracts 256 of 768)
CPAD = 208                     # padded center count (16B-aligned fp8 stride)
W = 32                         # label window width after per-core rotation
EPS_COS = 1e-8

_TRACE = False                 # test.py flips this for profiling runs
_TRACE_DIR = None
last_results = None

_built = {}                    # nt -> Bass


def _groups(nt):
    # small first group so the first matmul starts as early as possible;
    # remainder last so the tail chain is short
    if nt <= 2:
        return [nt]
    gs = [2]
    r = nt - 2
    gs += [4] * (r // 4)
    if r % 4:
        gs.append(r % 4)
    return gs


def _build(nt):
    """Device program for nt 128-row tiles per core."""
    if nt in _built:
        return _built[nt]
    nc = bass.Bass("TRN2", debug=False, num_devices=N_CORES)

    fp8 = mybir.dt.float8e4
    bf16 = mybir.dt.bfloat16
    f32 = mybir.dt.float32
    DR = mybir.MatmulPerfMode.DoubleRow

    # ft[p, t, s, b] = f_rows[t*128 + b, s*128 + p], fp8 (partition-major so
    # group DMAs iterate src and dst in the same order)
    ft = nc.dram_tensor("ft", [P, nt, 2 * KP, P], fp8, kind="ExternalInput")
    # cnt[p, s, c] = cn_rot[c, s*128 + p], fp8 (c in [200,208) zero)
    cnt = nc.dram_tensor("cnt", [P, 2 * KP, CPAD], fp8, kind="ExternalInput")
    # mask[p, t, j] = 2*labelled at the row's rotated label column, bf16
    msk = nc.dram_tensor("msk", [P, nt, W], bf16, kind="ExternalInput")
    out = nc.dram_tensor("out", [1, 1], f32, kind="ExternalOutput")

    with tile.TileContext(nc) as tc:
        gs = _groups(nt)
        with (
            tc.tile_pool(name="singles", bufs=1) as singles,
            tc.tile_pool(name="ftp", bufs=3) as ftp,
            tc.tile_pool(name="work", bufs=3) as work,
            tc.tile_pool(name="psum", bufs=3, space="PSUM") as psum,
            tc.tile_pool(name="psum1", bufs=1, space="PSUM") as psum1,
        ):
            # critical-path order on SP: ft group 0 first, then centers; the
            # mask rides the DVE queue (not needed until the first T-mult)
            ft0_sb = ftp.tile([P, gs[0], 2 * KP, P], fp8, tag="ft")
            nc.sync.dma_start(ft0_sb[:], ft[:, 0 : gs[0]])
            cnt_sb = singles.tile([P, 2 * KP, CPAD], fp8)
            nc.sync.dma_start(cnt_sb[:], cnt[:])
            msk_sb = singles.tile([P, nt, W], bf16)
            nc.scalar.dma_start(msk_sb[:], msk[:])

            s_all = singles.tile([P, nt], f32)
            t2_all = singles.tile([P, nt], f32)
            recip = singles.tile([P, nt], f32)
            term = singles.tile([P, nt], f32)
            ones_sb = singles.tile([P, 1], f32)
            nc.vector.memset(ones_sb[:], 1.0)

            t0 = 0
            for gi, gsz in enumerate(gs):
                if gi == 0:
                    ft_sb = ft0_sb
                else:
                    ft_sb = ftp.tile([P, gsz, 2 * KP, P], fp8, tag="ft")
                    nc.sync.dma_start(ft_sb[:], ft[:, t0 : t0 + gsz])

                # 256-elem pitch: each 208-wide tile slice stays in one bank
                cos_ps = psum.tile([P, gsz, 256], f32, tag="cos")
                for j in range(gsz):
                    for kq in range(KP):
                        nc.tensor.matmul(
                            cos_ps[:, j, 0:CPAD],
                            ft_sb[:, j, 2 * kq : 2 * kq + 2, :],
                            cnt_sb[:, 2 * kq : 2 * kq + 2, :],
                            start=(kq == 0),
                            stop=(kq == KP - 1),
                            perf_mode=DR,
                        )

                sl = slice(t0, t0 + gsz)
                # S: rowsum of |cos| straight from PSUM (reduce-with-abs)
                nc.vector.tensor_reduce(
                    out=s_all[:, sl],
                    in_=cos_ps[:, :, 0:C],
                    op=mybir.AluOpType.add,
                    axis=mybir.AxisListType.X,
                    apply_absolute_value=True,
                )
                # T': |cos| window on ACT, mask-mult on GpSimd, reduce on DVE
                win = work.tile([P, gsz, W], bf16, tag="win")
                nc.scalar.activation(
                    out=win[:],
                    in_=cos_ps[:, :, 0:W],
                    func=mybir.ActivationFunctionType.Abs,
                )
                tt = work.tile([P, gsz, W], bf16, tag="tt")
                nc.gpsimd.tensor_tensor(
                    out=tt[:], in0=win[:], in1=msk_sb[:, sl, :],
                    op=mybir.AluOpType.mult,
                )
                nc.vector.tensor_reduce(
                    out=t2_all[:, sl],
                    in_=tt[:],
                    op=mybir.AluOpType.add,
                    axis=mybir.AxisListType.X,
                )
                # per-group epilogue keeps the end-of-kernel chain short
                nc.vector.reciprocal(recip[:, sl], s_all[:, sl])
                nc.gpsimd.tensor_tensor(
                    out=term[:, sl], in0=t2_all[:, sl], in1=recip[:, sl],
                    op=mybir.AluOpType.mult,
                )
                t0 += gsz

            # collapse to one scalar on-chip: single 4B store
            tot_ps = psum1.tile([1, nt], f32)
            nc.tensor.matmul(tot_ps[:], ones_sb[:], term[:], start=True, stop=True)
            out_sb = singles.tile([1, 1], f32)
            nc.vector.tensor_reduce(
                out=out_sb[:], in_=tot_ps[:], op=mybir.AluOpType.add,
                axis=mybir.AxisListType.X,
            )
            nc.sync.dma_start(out[:], out_sb[:])

    _built[nt] = nc
    return nc


def kernel(features, centers, labels, labelled_or_not):
    global last_results

    f8 = ml_dtypes.float8_e4m3
    bf = ml_dtypes.bfloat16
    features = np.asarray(features, dtype=np.float32)
    centers = np.asarray(centers, dtype=np.float32)
    labels_i = np.asarray(labels).astype(np.int64)
    lab_b = np.asarray(labelled_or_not).astype(bool)

    n_lab = int(lab_b.sum())
    assert n_lab > 0

    # keep only labelled rows; sort them by label
    f_l = features[lab_b]
    l_l = labels_i[lab_b]
    order = np.argsort(l_l, kind="stable")
    f_s = f_l[order]
    l_s = l_l[order]

    # near-equal per-core real-row counts (padding spread over all cores so
    # each core's label span stays well under W)
    base, rem = divmod(n_lab, N_CORES)
    counts = [base + (1 if c < rem else 0) for c in range(N_CORES)]
    nt = max(1, -(-counts[0] // P))
    rows_core = nt * P
    starts = np.cumsum([0] + counts)

    # normalized centers (host, O(C*D))
    cn = centers / np.maximum(
        np.linalg.norm(centers, axis=1, keepdims=True), EPS_COS
    )

    in_maps = []
    for c in range(N_CORES):
        nreal = counts[c]
        fc = f_s[starts[c] : starts[c] + nreal]
        lc = l_s[starts[c] : starts[c] + nreal]
        pad = rows_core - nreal
        if pad:
            # repeat last real row (finite S) with mask weight 0
            fc = np.concatenate([fc, np.repeat(fc[-1:], pad, axis=0)])
            lc = np.concatenate([lc, np.repeat(lc[-1:], pad)])
        wc = np.zeros(rows_core, dtype=np.float32)
        wc[:nreal] = 2.0

        # rotate centers so this core's labels land in columns [0, W)
        l0 = int(lc[0])
        rel = (lc - l0) % C
        assert rel.max() < W, f"core {c} label span {rel.max()} >= {W}"
        cn_rot = cn[(l0 + np.arange(C)) % C]
        cnt_full = np.zeros((CPAD, D), dtype=np.float32)
        cnt_full[:C] = cn_rot
        cnt_host = np.ascontiguousarray(
            cnt_full.reshape(CPAD, 2 * KP, P).transpose(2, 1, 0).astype(f8)
        )

        # ft[p, t, s, b] = fc[t*128 + b, s*128 + p]
        ft_host = np.ascontiguousarray(
            fc.reshape(nt, P, 2 * KP, P).transpose(3, 0, 2, 1).astype(f8)
        )

        # mask[p, t, j]: 2*labelled at the rotated label column
        mask_flat = np.zeros((rows_core, W), dtype=np.float32)
        mask_flat[np.arange(rows_core), rel] = wc
        msk_host = np.ascontiguousarray(
            mask_flat.reshape(nt, P, W).transpose(1, 0, 2).astype(bf)
        )

        in_maps.append({"ft": ft_host, "cnt": cnt_host, "msk": msk_host})

    nc = _build(nt)
    kwargs = {}
    if _TRACE:
        kwargs["trace"] = True
        if _TRACE_DIR:
            kwargs["tmpdir"] = _TRACE_DIR
    res = run_bass_kernel_spmd(nc, in_maps, core_ids=list(range(N_CORES)), **kwargs)
    last_results = res

    total = 0.0
    for c in range(N_CORES):
        total += float(res.results[c]["out"][0, 0])
    # loss = -sum_labelled (2T - S)/S = N_labelled - sum 2T/S
    return np.array(float(n_lab) - total, dtype=np.float32)


# revision 16
# speedup vs baseline: 1.6577x; 1.0187x over previous
"""Trainium2 Bass kernel for the labelled contrastive loss.

Math (per batch row b, label L, over C=200 centers):
    cos[b,c] = <f_b, c_c> / (|f_b| |c_c|)
    a = |cos|;  S_b = sum_c a[b,c];  row term = (2*a[b,L_b] - S_b)/S_b
    loss = -sum over labelled rows of row term
Two exact host-side reductions shrink the device problem:
  1. |f_b| cancels in the ratio -> never computed; centers are normalized
     on host (O(C*D)) and the kernel works on raw = f @ cn^T.
  2. loss = N_labelled - sum_labelled 2*T_b/S_b, and UNLABELLED ROWS
     CONTRIBUTE NOTHING -> the kernel only processes the ~B/2 labelled
     rows; N_labelled is counted on host.

Device layout tricks:
  - fp8(e4m3) features/centers with perf_mode=DoubleRow matmuls (2 k-planes
    per pass, fp8 double-pumped PE). C padded 200->208 zero centers so the
    DoubleRow moving-AP stride is 16B-aligned; the pad columns produce
    cos=0 which add nothing to S.
  - Labelled rows are SORTED BY LABEL on host and each core's centers are
    ROTATED so that core's label range maps to columns [0, 32). T-extraction
    is then one batched 32-wide masked reduce per PSUM quad instead of a
    200-wide one-hot per tile; the mask (with 2*labelled folded in) is host
    precomputed. Zero-pad rows duplicate the last real row (finite S) with
    mask 0.
  - Per 4-tile PSUM quad (2 banks): 12 DoubleRow matmuls -> one ACT Abs
    (f32 PSUM -> bf16 SBUF) -> DVE rowsum for S + masked window reduce for
    T'. Epilogue: T'*(1/S), ones-matmul partition collapse, single 4B store.
"""

import math

import numpy as np
import ml_dtypes

import concourse.bass as bass
import concourse.tile as tile
from concourse import mybir
from concourse.bass_utils import run_bass_kernel_spmd

# ---------------------------------------------------------------------------
# Workaround for walrus "Too many sync wait commands": this toolchain only
# encodes a limited number of sem waits per instruction, so spread excess
# waits over preceding same-engine nops — both for scheduled instructions
# (pre-lowering pass) and for the TileContext tail drain.
# ---------------------------------------------------------------------------
from concourse.vector_clock import ScopedClock

_MAX_WAITS = 1
_split_counter = [0]


def _split_waits_in_ordered(ordered):
    for bb_name, insts in ordered.items():
        new = []
        for inst in insts:
            si = getattr(inst, "sync_info", None)
            waits = list(si.on_wait) if si is not None and si.on_wait else []
            if len(waits) > _MAX_WAITS:
                updates = list(si.on_update) if si.on_update else []
                head, tail = waits[:-_MAX_WAITS], waits[-_MAX_WAITS:]
                while head:
                    n = mybir.InstNoOp(
                        name=f"I-wsplit-{_split_counter[0]}", ins=[], outs=[]
                    )
                    _split_counter[0] += 1
                    n.engine = inst.engine
                    n.bass_nofuse = True
                    n.sync_info = mybir.SyncInfo(
                        on_wait=head[:_MAX_WAITS], on_update=[]
                    )
                    head = head[_MAX_WAITS:]
                    new.append(n)
                inst.sync_info = mybir.SyncInfo(on_wait=tail, on_update=updates)
            new.append(inst)
        ordered[bb_name] = new


_orig_lower_ordered = tile.TileContext._lower_ordered_insts


def _patched_lower_ordered(self, ordered):
    _split_waits_in_ordered(ordered)
    return _orig_lower_ordered(self, ordered)


tile.TileContext._lower_ordered_insts = _patched_lower_ordered

_ORIG_DRAIN = tile.TileContext._drain_and_barrier


def _patched_drain_and_barrier(self, tick_clock, wait_clock):
    """Minimal kernel tail replacing the stock drain + two EVSEM-butterfly
    barriers (~15us). See the original notes: SP nops carry the final sem
    waits, engines drain + bump a tail semaphore, GpSimd waits for them and
    range-clears tile semaphores/queue state; halting engines complete the
    NEFF with everything zeroed for re-execution.
    """
    nc = self.nc
    carrier = nc.sync.nop(nofuse=True)
    wait_clock.add_sem_waits(carrier.ins, ScopedClock({None: tick_clock.global_clock}))
    si = carrier.ins.sync_info
    waits = list(si.on_wait) if si is not None and si.on_wait else []
    if len(waits) > _MAX_WAITS:
        updates = list(si.on_update) if si.on_update else []
        carrier.ins.sync_info = mybir.SyncInfo(on_wait=[], on_update=updates)
        rest = waits
        while rest:
            n = nc.sync.nop(nofuse=True)
            n.ins.sync_info = mybir.SyncInfo(on_wait=rest[:_MAX_WAITS], on_update=[])
            rest = rest[_MAX_WAITS:]
    nc.sync.drain()

    tail_sem = nc.alloc_semaphore("tile_tail_sem")
    n_inc = 0
    for eng_type, eng in nc.engines.items():
        if eng_type == mybir.EngineType.Pool:
            continue
        eng.drain()
        eng.sem_inc(tail_sem, 1)
        n_inc += 1
    nc.gpsimd.drain()
    nc.gpsimd.wait_ge(tail_sem, n_inc)

    assert self.sems is not None
    popped = nc._tile_sem_poison_stack.pop()
    assert popped is self._sem_poison
    nc.clear_and_free_semaphores(list(self.sems.allocated().values()))
    nc.clear_and_free_semaphores([tail_sem])


tile.TileContext._drain_and_barrier = _patched_drain_and_barrier

# ---------------------------------------------------------------------------
# Problem constants (hardcoded per contract)
# ---------------------------------------------------------------------------
N_CORES = 8
B, D, C = 32768, 768, 200
P = 128                        # partitions
KP = 3                         # DoubleRow k passes (each contracts 256 of 768)
CPAD = 208                     # padded center count (16B-aligned fp8 stride)
W = 32                         # label window width after per-core rotation
EPS_COS = 1e-8

_TRACE = False                 # test.py flips this for profiling runs
_TRACE_DIR = None
last_results = None

_built = {}                    # nt -> Bass


def _groups(nt):
    # small first group so the first matmul starts as early as possible;
    # remainder last so the tail chain is short
    if nt <= 2:
        return [nt]
    gs = [2]
    r = nt - 2
    gs += [4] * (r // 4)
    if r % 4:
        gs.append(r % 4)
    return gs


def _build(nt):
    """Device program for nt 128-row tiles per core."""
    if nt in _built:
        return _built[nt]
    nc = bass.Bass("TRN2", debug=False, num_devices=N_CORES)

    fp8 = mybir.dt.float8e4
    bf16 = mybir.dt.bfloat16
    f32 = mybir.dt.float32
    DR = mybir.MatmulPerfMode.DoubleRow

    # ft[p, t, s, b] = f_rows[t*128 + b, s*128 + p], fp8 (partition-major so
    # group DMAs iterate src and dst in the same order)
    ft = nc.dram_tensor("ft", [P, nt, 2 * KP, P], fp8, kind="ExternalInput")
    # cnt[p, s, c] = cn_rot[c, s*128 + p], fp8 (c in [200,208) zero)
    cnt = nc.dram_tensor("cnt", [P, 2 * KP, CPAD], fp8, kind="ExternalInput")
    # mask[p, t, j] = 2*labelled at the row's rotated label column, bf16
    msk = nc.dram_tensor("msk", [P, nt, W], bf16, kind="ExternalInput")
    out = nc.dram_tensor("out", [1, 1], f32, kind="ExternalOutput")

    with tile.TileContext(nc) as tc:
        gs = _groups(nt)
        with (
            tc.tile_pool(name="singles", bufs=1) as singles,
            tc.tile_pool(name="work", bufs=3) as work,
            tc.tile_pool(name="psum", bufs=3, space="PSUM") as psum,
            tc.tile_pool(name="psum1", bufs=1, space="PSUM") as psum1,
        ):
            # all ft tiles stay resident (no pool rotation gating the DMAs);
            # SP issue order: ft g0, ft g1, cnt, ft g2..; msk rides GpSimd
            ft_sbs = []
            ofs = 0
            for gi, gsz in enumerate(gs):
                ft_sbs.append(
                    singles.tile(
                        [P, gsz, 2 * KP, P], fp8, name=f"ft_sb{gi}", tag=f"ft{gi}"
                    )
                )
            nc.sync.dma_start(ft_sbs[0][:], ft[:, 0 : gs[0]])
            if len(gs) > 1:
                nc.sync.dma_start(ft_sbs[1][:], ft[:, gs[0] : gs[0] + gs[1]])
            cnt_sb = singles.tile([P, 2 * KP, CPAD], fp8)
            nc.sync.dma_start(cnt_sb[:], cnt[:])
            ofs = gs[0] + (gs[1] if len(gs) > 1 else 0)
            for gi in range(2, len(gs)):
                nc.sync.dma_start(ft_sbs[gi][:], ft[:, ofs : ofs + gs[gi]])
                ofs += gs[gi]
            msk_sb = singles.tile([P, nt, W], bf16)
            nc.gpsimd.dma_start(msk_sb[:], msk[:])

            s_all = singles.tile([P, nt], f32)
            t2_all = singles.tile([P, nt], f32)
            recip = singles.tile([P, nt], f32)
            term = singles.tile([P, nt], f32)
            ones_sb = singles.tile([P, 1], f32)
            nc.vector.memset(ones_sb[:], 1.0)
            # dummy Abs so the ACT table load happens in the idle preamble
            warm = singles.tile([P, 1], bf16)
            nc.scalar.activation(
                out=warm[:], in_=ones_sb[:],
                func=mybir.ActivationFunctionType.Abs,
            )

            t0 = 0
            for gi, gsz in enumerate(gs):
                ft_sb = ft_sbs[gi]
                last = gi == len(gs) - 1

                # 256-elem pitch: each 208-wide tile slice stays in one bank
                cos_ps = psum.tile([P, gsz, 256], f32, tag="cos")
                for j in range(gsz):
                    for kq in range(KP):
                        nc.tensor.matmul(
                            cos_ps[:, j, 0:CPAD],
                            ft_sb[:, j, 2 * kq : 2 * kq + 2, :],
                            cnt_sb[:, 2 * kq : 2 * kq + 2, :],
                            start=(kq == 0),
                            stop=(kq == KP - 1),
                            perf_mode=DR,
                        )

                sl = slice(t0, t0 + gsz)
                # S: rowsum of |cos| straight from PSUM (reduce-with-abs)
                nc.vector.tensor_reduce(
                    out=s_all[:, sl],
                    in_=cos_ps[:, :, 0:C],
                    op=mybir.AluOpType.add,
                    axis=mybir.AxisListType.X,
                    apply_absolute_value=True,
                )
                nc.vector.reciprocal(recip[:, sl], s_all[:, sl])
                if not last:
                    # T': |cos| window on ACT, mask-mult on GpSimd, reduce DVE
                    win = work.tile([P, gsz, W], bf16, tag="win")
                    nc.scalar.activation(
                        out=win[:],
                        in_=cos_ps[:, :, 0:W],
                        func=mybir.ActivationFunctionType.Abs,
                    )
                    tt = work.tile([P, gsz, W], bf16, tag="tt")
                    nc.gpsimd.tensor_tensor(
                        out=tt[:], in0=win[:], in1=msk_sb[:, sl, :],
                        op=mybir.AluOpType.mult,
                    )
                    nc.vector.tensor_reduce(
                        out=t2_all[:, sl],
                        in_=tt[:],
                        op=mybir.AluOpType.add,
                        axis=mybir.AxisListType.X,
                    )
                    nc.gpsimd.tensor_tensor(
                        out=term[:, sl], in0=t2_all[:, sl], in1=recip[:, sl],
                        op=mybir.AluOpType.mult,
                    )
                else:
                    # last group: pure-DVE T-path — |cos*m| = |cos|*m, so one
                    # masked mult from PSUM + reduce-with-abs; no cross-engine
                    # hops on the end-of-kernel chain
                    tt = work.tile([P, gsz, W], f32, tag="ttl")
                    nc.vector.tensor_tensor(
                        out=tt[:], in0=cos_ps[:, :, 0:W], in1=msk_sb[:, sl, :],
                        op=mybir.AluOpType.mult,
                    )
                    nc.vector.tensor_reduce(
                        out=t2_all[:, sl],
                        in_=tt[:],
                        op=mybir.AluOpType.add,
                        axis=mybir.AxisListType.X,
                        apply_absolute_value=True,
                    )
                    nc.vector.tensor_tensor(
                        out=term[:, sl], in0=t2_all[:, sl], in1=recip[:, sl],
                        op=mybir.AluOpType.mult,
                    )
                t0 += gsz

            # collapse to one scalar on-chip: single 4B store
            tot_ps = psum1.tile([1, nt], f32)
            nc.tensor.matmul(tot_ps[:], ones_sb[:], term[:], start=True, stop=True)
            out_sb = singles.tile([1, 1], f32)
            nc.vector.tensor_reduce(
                out=out_sb[:], in_=tot_ps[:], op=mybir.AluOpType.add,
                axis=mybir.AxisListType.X,
            )
            nc.sync.dma_start(out[:], out_sb[:])

    _built[nt] = nc
    return nc


def kernel(features, centers, labels, labelled_or_not):
    global last_results

    f8 = ml_dtypes.float8_e4m3
    bf = ml_dtypes.bfloat16
    features = np.asarray(features, dtype=np.float32)
    centers = np.asarray(centers, dtype=np.float32)
    labels_i = np.asarray(labels).astype(np.int64)
    lab_b = np.asarray(labelled_or_not).astype(bool)

    n_lab = int(lab_b.sum())
    assert n_lab > 0

    # keep only labelled rows; sort them by label
    f_l = features[lab_b]
    l_l = labels_i[lab_b]
    order = np.argsort(l_l, kind="stable")
    f_s = f_l[order]
    l_s = l_l[order]

    # near-equal per-core real-row counts (padding spread over all cores so
    # each core's label span stays well under W)
    base, rem = divmod(n_lab, N_CORES)
    counts = [base + (1 if c < rem else 0) for c in range(N_CORES)]
    nt = max(1, -(-counts[0] // P))
    rows_core = nt * P
    starts = np.cumsum([0] + counts)

    # normalized centers (host, O(C*D))
    cn = centers / np.maximum(
        np.linalg.norm(centers, axis=1, keepdims=True), EPS_COS
    )

    in_maps = []
    for c in range(N_CORES):
        nreal = counts[c]
        fc = f_s[starts[c] : starts[c] + nreal]
        lc = l_s[starts[c] : starts[c] + nreal]
        pad = rows_core - nreal
        if pad:
            # repeat last real row (finite S) with mask weight 0
            fc = np.concatenate([fc, np.repeat(fc[-1:], pad, axis=0)])
            lc = np.concatenate([lc, np.repeat(lc[-1:], pad)])
        wc = np.zeros(rows_core, dtype=np.float32)
        wc[:nreal] = 2.0

        # rotate centers so this core's labels land in columns [0, W)
        l0 = int(lc[0])
        rel = (lc - l0) % C
        assert rel.max() < W, f"core {c} label span {rel.max()} >= {W}"
        cn_rot = cn[(l0 + np.arange(C)) % C]
        cnt_full = np.zeros((CPAD, D), dtype=np.float32)
        cnt_full[:C] = cn_rot
        cnt_host = np.ascontiguousarray(
            cnt_full.reshape(CPAD, 2 * KP, P).transpose(2, 1, 0).astype(f8)
        )

        # ft[p, t, s, b] = fc[t*128 + b, s*128 + p]
        ft_host = np.ascontiguousarray(
            fc.reshape(nt, P, 2 * KP, P).transpose(3, 0, 2, 1).astype(f8)
        )

        # mask[p, t, j]: 2*labelled at the rotated label column
        mask_flat = np.zeros((rows_core, W), dtype=np.float32)
        mask_flat[np.arange(rows_core), rel] = wc
        msk_host = np.ascontiguousarray(
            mask_flat.reshape(nt, P, W).transpose(1, 0, 2).astype(bf)
        )

        in_maps.append({"ft": ft_host, "cnt": cnt_host, "msk": msk_host})

    nc = _build(nt)
    kwargs = {}
    if _TRACE:
        kwargs["trace"] = True
        if _TRACE_DIR:
            kwargs["tmpdir"] = _TRACE_DIR
    res = run_bass_kernel_spmd(nc, in_maps, core_ids=list(range(N_CORES)), **kwargs)
    last_results = res

    total = 0.0
    for c in range(N_CORES):
        total += float(res.results[c]["out"][0, 0])
    # loss = -sum_labelled (2T - S)/S = N_labelled - sum 2T/S
    return np.array(float(n_lab) - total, dtype=np.float32)
